# revision 13
# baseline (speedup 1.0000x reference)
"""Trainium2 Bass kernel for multi-head attention (b=4, n=2048, dim=256, H=8, D=32).

Sharding: 8 cores = 4 batches x 2 query-halves. Each core computes the full
attention for its 1024 query rows against all 2048 keys of its batch.
No collectives; host slices inputs (cast to fp16) and concatenates outputs.

Per-core dataflow:
  c1s [1024,256], c2b [2048,256] fp16 --XBAR DMA transpose--> c1T, c2T (fp16)
  qT = Wq^T c1^T  [256,1024]  (fp16 matmul, f32r storage; head h at 32h%128)
  kT = Wk^T c2^T  [256,2048]
  v  = c2 @ Wv -> v4 fp16 [128, kb, h, 33]  (ones column fused per head)
  per unit (kb, qh): S^T = kT_h^T x qT_h  (f32r, PSUM-write-bound 1 col/cyc)
                     P^T = exp(0.125*S^T) -> fp16  (ACT)
  AV: two heads on disjoint PE column tiles (cols 0-63 / 64-127) so their
      rhs streams run concurrently on separate XBUSes (~1.4x aggregate)
  normalize: recip_approx_fast(denominator) via SBUF (bit-trick ops cannot
      read PE-accumulated PSUM), DMA broadcast, DVE multiply into out_sb2
      with 4 heads stacked per 128 partitions
  y = out @ Wo: K=128 contraction (4 heads at once), 2 matmuls per q-block
"""

import os
import sys

for p in ("/opt/trn_rl_repo", "/opt/pypackages"):
    if p not in sys.path:
        sys.path.insert(0, p)

from contextlib import ExitStack

import numpy as np

import concourse.bass as bass
import concourse.bacc as bacc
import concourse.mybir as mybir
import concourse.tile as tile

P = 128
NQ = 1024          # per-core query rows
NK = 2048          # keys
DIM = 256
H = 8
D = 32
SCALE = 64 ** -0.5  # 0.125, matches reference
FP32 = mybir.dt.float32
F32R = mybir.dt.float32r
FP16 = mybir.dt.float16

N_CORES = 8


def _strip_pe_self_waits(nc):
    """Drop PE-sem waits from PE matmuls. The PE is strictly in-order with a
    single PSUM write port and never reads PSUM nor writes SBUF, so a PE
    instruction can never race another PE instruction; Tile still emits these
    same-engine waits, and matmul instructions only support one sync wait."""
    pe = mybir.EngineType.PE
    for f in nc.m.functions:
        for bb in f.blocks:
            for inst in bb.instructions:
                if type(inst).__name__ != "InstMatmult" or inst.engine != pe:
                    continue
                si = inst.sync_info
                if si is None:
                    continue
                ws = [w for w in si.on_wait if not str(w.ant_name).startswith("PE_")]
                if len(ws) != len(si.on_wait):
                    si.on_wait = ws
                    inst.sync_info = si


def _strip_redundant_waits(nc):
    """ACT is also strictly in-order: drop Activation-sem self-waits from
    ACTIVATE instructions (WAW on cycled SBUF output slots is FIFO-safe).
    Output stores: drop DMAHW lane-bookkeeping waits (they only order the
    store against an unrelated earlier input DMA that reused the same
    round-robin completion lane; the data dependency is the DVE wait)."""
    act = mybir.EngineType.Activation
    store_names = set(getattr(nc, "_y_store_names", ()))
    for f in nc.m.functions:
        for bb in f.blocks:
            for inst in bb.instructions:
                si = getattr(inst, "sync_info", None)
                if si is None or len(si.on_wait) <= 1:
                    continue
                tn = type(inst).__name__
                if tn == "InstActivation" and inst.engine == act:
                    ws = [w for w in si.on_wait
                          if not str(w.ant_name).startswith("Activation")]
                elif tn == "InstDMACopy" and inst.name in store_names:
                    ws = [w for w in si.on_wait
                          if not str(w.ant_name).startswith("DMAHW")]
                else:
                    continue
                if len(ws) != len(si.on_wait):
                    si.on_wait = ws
                    inst.sync_info = si


def build_nc():
    nc = bacc.Bacc()
    c1s = nc.dram_tensor("c1s", [NQ, DIM], FP16, kind="ExternalInput")
    c2b = nc.dram_tensor("c2b", [NK, DIM], FP16, kind="ExternalInput")
    wq = nc.dram_tensor("wq", [DIM, DIM], FP32, kind="ExternalInput")
    wk = nc.dram_tensor("wk", [DIM, DIM], FP32, kind="ExternalInput")
    wv = nc.dram_tensor("wv", [DIM, DIM], FP32, kind="ExternalInput")
    wo = nc.dram_tensor("wo", [DIM, DIM], FP32, kind="ExternalInput")
    bo = nc.dram_tensor("bo", [DIM], FP32, kind="ExternalInput")
    y = nc.dram_tensor("y", [NQ, DIM], FP32, kind="ExternalOutput")
    rdd = nc.dram_tensor("rdd", [4, 2, NQ], FP32)

    with tile.TileContext(nc) as tc, ExitStack() as ctx:
        _body(tc, ctx, c1s, c2b, wq, wk, wv, wo, bo, y, rdd)
    if os.environ.get("KERNEL_STRIP_WAITS", "1") == "1":
        _strip_pe_self_waits(nc)
        _strip_redundant_waits(nc)
    nc.finalize()
    return nc


def _body(tc, ctx, c1s, c2b, wq, wk, wv, wo, bo, y, rdd):
    nc = tc.nc
    Exp = mybir.ActivationFunctionType.Exp
    MULT = mybir.AluOpType.mult
    ADD = mybir.AluOpType.add

    persist = ctx.enter_context(tc.tile_pool(name="persist", bufs=1))
    stage = ctx.enter_context(tc.tile_pool(name="stage", bufs=1))

    # ---- persistent activations ----
    c1T = [persist.tile([P, NQ], FP16, tag=f"c1T{i}", name=f"c1T{i}") for i in range(2)]
    c2T = [persist.tile([P, NK], FP16, tag=f"c2T{i}", name=f"c2T{i}") for i in range(2)]
    qT = [persist.tile([P, NQ], F32R, tag=f"qT{i}", name=f"qT{i}") for i in range(2)]
    kT = [persist.tile([P, NK], F32R, tag=f"kT{i}", name=f"kT{i}") for i in range(2)]
    # v with fused ones column: [128, kb, h, 33]
    v4 = persist.tile([P, NK // P, H, D + 1], FP16, tag="v4")
    # normalized per-head outputs: 4 heads stacked per 128 partitions:
    # out_sb2[(h%4)*32 + d, h//4, q]
    out_sb2 = persist.tile([P, 2, NQ], FP16, tag="out_sb2")

    # ---- input transposes straight off DRAM via the XBAR DMA ----
    # c2T first: kt_proj/v_proj are the earliest consumers
    for fh in range(2):
        nc.sync.dma_start_transpose(out=c2T[fh], in_=c2b[:, fh * P:(fh + 1) * P])
    for fh in range(2):
        nc.sync.dma_start_transpose(out=c1T[fh], in_=c1s[:, fh * P:(fh + 1) * P])

    # ---- weights ----
    wq_sb = persist.tile([P, 2, DIM], FP16, tag="wq")
    wk_sb = persist.tile([P, 2, DIM], FP16, tag="wk")
    wv_sb = persist.tile([P, 2, DIM], FP16, tag="wv")
    # Wo for K=128 head-stacked contraction: wo4[p, hg, f] = Wo[hg*128+p, f]
    wo4 = persist.tile([P, 2, DIM], FP16, tag="wo4")
    for wi, (w_dram, w_sb) in enumerate(((wq, wq_sb), (wk, wk_sb), (wv, wv_sb),
                                         (wo, wo4))):
        wst = stage.tile([P, 2, DIM], FP32, tag=f"wst{wi}", name=f"wst{wi}")
        nc.sync.dma_start(out=wst, in_=w_dram.rearrange("(c p) f -> p c f", p=P))
        nc.vector.tensor_copy(out=w_sb, in_=wst)
    # bias broadcast to all partitions (staged through DVE like the weights)
    bo_st = stage.tile([P, DIM], FP32, tag="bo_st")
    nc.gpsimd.dma_start(out=bo_st, in_=bo[:].partition_broadcast(P))
    bo_bc = persist.tile([P, DIM], FP32, tag="bo")
    nc.vector.tensor_copy(out=bo_bc, in_=bo_st)
    # warm the ACT exp table while the prologue runs
    exp_warm = persist.tile([1, 4], FP16, tag="exp_warm")
    nc.scalar.activation(out=exp_warm, in_=bo_bc[0:1, 0:4],
                         func=Exp, scale=float(SCALE))
    pt_pool = ctx.enter_context(tc.tile_pool(name="pt", bufs=6))
    small1 = ctx.enter_context(tc.tile_pool(name="small1", bufs=1))
    yout = ctx.enter_context(tc.tile_pool(name="yout", bufs=8))
    dn_pool = ctx.enter_context(tc.tile_pool(name="dn", bufs=2))

    with tc.tile_pool(name="st_psum", bufs=2, space="PSUM") as st_psum, \
         tc.tile_pool(name="av_psum", bufs=2, space="PSUM") as av_psum:

        def qt_proj(fb, qb):
            pp = st_psum.tile([P, 1024], FP32, tag="st", name="pp")[:, :512]
            for c in range(2):
                nc.tensor.matmul(
                    pp, lhsT=wq_sb[:, c, fb * P:(fb + 1) * P],
                    rhs=c1T[c][:, qb * 512:(qb + 1) * 512],
                    start=(c == 0), stop=(c == 1),
                )
            nc.vector.tensor_copy(out=qT[fb][:, qb * 512:(qb + 1) * 512], in_=pp)

        def kt_proj(fb, nb):
            pp = st_psum.tile([P, 1024], FP32, tag="st", name="pp")[:, :512]
            for c in range(2):
                nc.tensor.matmul(
                    pp, lhsT=wk_sb[:, c, fb * P:(fb + 1) * P],
                    rhs=c2T[c][:, nb * 512:(nb + 1) * 512],
                    start=(c == 0), stop=(c == 1),
                )
            nc.vector.tensor_copy(out=kT[fb][:, nb * 512:(nb + 1) * 512], in_=pp)

        def v_proj(kb):
            pp = st_psum.tile([P, 1024], FP32, tag="st", name="pp")[:, :512]
            for c in range(2):
                nc.tensor.matmul(
                    pp[:, :DIM], lhsT=c2T[c][:, kb * P:(kb + 1) * P],
                    rhs=wv_sb[:, c, :], start=(c == 0), stop=(c == 1),
                )
            nc.vector.tensor_copy(
                out=v4[:, kb, :, 0:D],
                in_=pp[:, :DIM].rearrange("p (h d) -> p h d", d=D),
            )

        nc.gpsimd.memset(v4, 1.0)  # ones column; v copies overwrite cols 0..D-1

        # minimal pre-pair-0 prologue: exactly what pair 0's first units need
        qt_proj(0, 0)
        qt_proj(0, 1)
        kt_proj(0, 0)

        # deadline-scheduled leftover prologue work, injected into pair-0 units
        extras = {}

        def sched(u, fn, *a):
            extras.setdefault(u, []).append((fn, a))

        for kb in range(16):
            sched(2 * kb, v_proj, kb)             # needed by AV at unit 2*kb+2
        for nb in (1, 2, 3):
            sched(8 * nb - 4, kt_proj, 0, nb)     # needed by S^T kb=4nb (unit 8nb)
        sched(20, qt_proj, 1, 0)
        sched(21, qt_proj, 1, 1)
        for nb in range(4):
            sched(22 + nb, kt_proj, 1, nb)

        # ---- attention: head pairs; row-packed S^T, col-tiled AV ----
        for pr in range(4):
            h0 = 2 * pr
            ht = h0 // 4
            b0, b1 = (h0 % 4) * 32, (h0 % 4) * 32 + 32
            av = av_psum.tile([64 + D + 1, NQ], FP32, tag="av")
            pending = []

            def emit_av(ent):
                pt, kb, qh = ent
                for e in range(2):
                    nc.tensor.matmul(
                        av[64 * e:64 * e + D + 1, qh * 512:(qh + 1) * 512],
                        lhsT=v4[:, kb, h0 + e, :],
                        rhs=pt[:, e * 512:(e + 1) * 512],
                        start=(kb == 0), stop=(kb == NK // P - 1),
                        skip_group_check=True,
                    )

            units = [(kb, qh) for kb in range(NK // P) for qh in range(NQ // 512)]
            for u, (kb, qh) in enumerate(units):
                if pr == 0:
                    for fn, a in extras.get(u, []):
                        fn(*a)
                lhsT0 = kT[ht][b0:b0 + 32, kb * P:(kb + 1) * P]
                lhsT1 = kT[ht][b1:b1 + 32, kb * P:(kb + 1) * P]
                qs = slice(qh * 512, (qh + 1) * 512)
                st = st_psum.tile([P, 1024], FP32, tag="st")
                nc.tensor.matmul(
                    st[:, 0:512], lhsT=lhsT0, rhs=qT[ht][b0:b0 + 32, qs],
                    start=True, stop=True, tile_position=(b0, 0),
                )
                nc.tensor.matmul(
                    st[:, 512:1024], lhsT=lhsT1, rhs=qT[ht][b1:b1 + 32, qs],
                    start=True, stop=True, tile_position=(b1, 0),
                )
                pt = pt_pool.tile([P, 1024], FP16, tag="pt")
                nc.scalar.activation(out=pt, in_=st, func=Exp, scale=float(SCALE))
                pending.append((pt, kb, qh))
                if len(pending) > 2:
                    emit_av(pending.pop(0))
            for ent in pending:
                emit_av(ent)

            # ---- normalize: 1/denominator via SBUF (bit-trick ops cannot
            # read PE-accumulated PSUM), DMA broadcast, DVE multiply ----
            dn_sb = dn_pool.tile([1, 2 * NQ], FP32, tag="dn", name="dn")
            for e in range(2):
                nc.vector.tensor_copy(
                    out=dn_sb[:, e * NQ:(e + 1) * NQ],
                    in_=av[64 * e + D:64 * e + D + 1, :])
            nc.vector.reciprocal_approx_fast(out=dn_sb, in_=dn_sb)
            nc.gpsimd.dma_start(out=rdd[pr], in_=dn_sb)
            bc_sb = small1.tile([32, 2 * NQ], FP32, tag="bcs", name="bcs")
            nc.gpsimd.dma_start(
                out=bc_sb,
                in_=rdd[pr].rearrange("two q -> (two q)").partition_broadcast(32),
            )
            for e in range(2):
                h = h0 + e
                hb = (h % 4) * 32
                for qh in range(NQ // 512):
                    qs = slice(qh * 512, (qh + 1) * 512)
                    nc.vector.tensor_tensor(
                        out=out_sb2[hb:hb + 32, h // 4, qs],
                        in0=av[64 * e:64 * e + D, qs],
                        in1=bc_sb[:, e * NQ + qh * 512:e * NQ + (qh + 1) * 512],
                        op=MULT,
                    )

    # ---- output projection + bias: K=128 contraction, 4 heads at once ----
    with tc.tile_pool(name="y_psum", bufs=2, space="PSUM") as y_psum:
        for qb in range(NQ // P):
            yp = y_psum.tile([P, 512], FP32, tag="y")
            for hg in range(2):
                nc.tensor.matmul(
                    yp[:, :DIM],
                    lhsT=out_sb2[:, hg, qb * P:(qb + 1) * P],
                    rhs=wo4[:, hg, :],
                    start=(hg == 0), stop=(hg == 1),
                )
            ys = yout.tile([P, DIM], FP32, tag="ys")
            nc.vector.tensor_tensor(out=ys, in0=yp[:, :DIM], in1=bo_bc, op=ADD)
            st_inst = nc.sync.dma_start(out=y[qb * P:(qb + 1) * P, :], in_=ys)
            nc._y_store_names = getattr(nc, "_y_store_names", []) + [st_inst.ins.name]


_NC_CACHE = None


def _get_nc():
    global _NC_CACHE
    if _NC_CACHE is None:
        _NC_CACHE = build_nc()
    return _NC_CACHE


def make_in_maps(c2, c1, Wq, Wk, Wv, Wo, bo):
    c1 = np.asarray(c1, np.float32).astype(np.float16)
    c2 = np.asarray(c2, np.float32).astype(np.float16)
    Wq, Wk, Wv, Wo, bo = (np.asarray(a, np.float32) for a in (Wq, Wk, Wv, Wo, bo))
    in_maps = []
    for core in range(N_CORES):
        b, qh = core // 2, core % 2
        in_maps.append({
            "c1s": np.ascontiguousarray(c1[b, qh * NQ:(qh + 1) * NQ, :]),
            "c2b": np.ascontiguousarray(c2[b]),
            "wq": Wq, "wk": Wk, "wv": Wv, "wo": Wo, "bo": bo,
        })
    return in_maps


def assemble(results):
    out = np.empty((4, 2 * NQ, DIM), np.float32)
    for core in range(N_CORES):
        b, qh = core // 2, core % 2
        out[b, qh * NQ:(qh + 1) * NQ, :] = results[core]["y"]
    return out


def run_spmd(inputs, trace=False, **kwargs):
    from concourse.bass_utils import run_bass_kernel_spmd

    nc = _get_nc()
    in_maps = make_in_maps(**inputs)
    res = run_bass_kernel_spmd(
        nc, in_maps, core_ids=list(range(N_CORES)), trace=trace, **kwargs
    )
    return assemble(res.results), res


def kernel(c2, c1, Wq, Wk, Wv, Wo, bo):
    out, _ = run_spmd(dict(c2=c2, c1=c1, Wq=Wq, Wk=Wk, Wv=Wv, Wo=Wo, bo=bo))
    return out


# revision 15
# speedup vs baseline: 1.0512x; 1.0512x over previous
"""Trainium2 Bass kernel for multi-head attention (b=4, n=2048, dim=256, H=8, D=32).

Sharding: 8 cores = 4 batches x 2 query-halves. Each core computes the full
attention for its 1024 query rows against all 2048 keys of its batch.
No collectives; host slices inputs and concatenates outputs.

Per-core dataflow (f32r storage, bf16 probabilities/values):
  c1s [1024,256], c2b [2048,256] --PE transpose--> c1T [256,1024], c2T [256,2048]
  qT = Wq^T c1^T  [256,1024]   (features on partitions; head h at 32h%128)
  kT = Wk^T c2^T  [256,2048]
  v  = c2 @ Wv    [2048, 8x(32+1)]  (keys on partitions; ones column per head)
  per unit (kb, qh): S^T = kT_h^T x qT_h  (f32r, PSUM-write-bound 1 col/cyc)
                     P^T = exp(0.125*S^T) -> bf16  (ACT)
  AV: two heads on disjoint PE column tiles (cols 0-63 / 64-127) so their
      rhs streams run concurrently on separate XBUSes (~1.4x aggregate)
  normalize: denominator rows -> SBUF (bit-trick DVE ops cannot read
      PE-accumulated PSUM), one reciprocal_approx_fast, DMA broadcast,
      2 DVE multiplies into out_sb2 (4 heads stacked per 128 partitions)
  y = out @ Wo: K=128 contraction (4 heads at once), 2 matmuls per q-block
"""

import os
import sys

for p in ("/opt/trn_rl_repo", "/opt/pypackages"):
    if p not in sys.path:
        sys.path.insert(0, p)

from contextlib import ExitStack

import numpy as np

import concourse.bass as bass
import concourse.bacc as bacc
import concourse.mybir as mybir
import concourse.tile as tile
from concourse.masks import make_identity

P = 128
NQ = 1024          # per-core query rows
NK = 2048          # keys
DIM = 256
H = 8
D = 32
SCALE = 64 ** -0.5  # 0.125, matches reference
FP32 = mybir.dt.float32
F32R = mybir.dt.float32r
BF16 = mybir.dt.bfloat16

N_CORES = 8


def _strip_pe_self_waits(nc):
    """Drop PE-sem waits from PE matmuls. The PE is strictly in-order with a
    single PSUM write port and never reads PSUM nor writes SBUF, so a PE
    instruction can never race another PE instruction; Tile still emits these
    same-engine waits, and matmul instructions only support one sync wait."""
    pe = mybir.EngineType.PE
    for f in nc.m.functions:
        for bb in f.blocks:
            for inst in bb.instructions:
                if type(inst).__name__ != "InstMatmult" or inst.engine != pe:
                    continue
                si = inst.sync_info
                if si is None:
                    continue
                ws = [w for w in si.on_wait if not str(w.ant_name).startswith("PE_")]
                if len(ws) != len(si.on_wait):
                    si.on_wait = ws
                    inst.sync_info = si


def _strip_redundant_waits(nc):
    """ACT is also strictly in-order: drop Activation-sem self-waits from
    ACTIVATE instructions (WAW on cycled SBUF output slots is FIFO-safe).
    Output stores: drop DMAHW lane-bookkeeping waits (they only order the
    store against an unrelated earlier input DMA that reused the same
    round-robin completion lane; the data dependency is the DVE wait)."""
    act = mybir.EngineType.Activation
    store_names = set(getattr(nc, "_y_store_names", ()))
    for f in nc.m.functions:
        for bb in f.blocks:
            for inst in bb.instructions:
                si = getattr(inst, "sync_info", None)
                if si is None or len(si.on_wait) <= 1:
                    continue
                tn = type(inst).__name__
                if tn == "InstActivation" and inst.engine == act:
                    ws = [w for w in si.on_wait
                          if not str(w.ant_name).startswith("Activation")]
                elif tn == "InstDMACopy" and inst.name in store_names:
                    ws = [w for w in si.on_wait
                          if not str(w.ant_name).startswith("DMAHW")]
                else:
                    continue
                if len(ws) != len(si.on_wait):
                    si.on_wait = ws
                    inst.sync_info = si


def build_nc():
    nc = bacc.Bacc()
    c1s = nc.dram_tensor("c1s", [NQ, DIM], F32R, kind="ExternalInput")
    c2b = nc.dram_tensor("c2b", [NK, DIM], F32R, kind="ExternalInput")
    wq = nc.dram_tensor("wq", [DIM, DIM], FP32, kind="ExternalInput")
    wk = nc.dram_tensor("wk", [DIM, DIM], FP32, kind="ExternalInput")
    wv = nc.dram_tensor("wv", [DIM, DIM], FP32, kind="ExternalInput")
    wo = nc.dram_tensor("wo", [DIM, DIM], FP32, kind="ExternalInput")
    bo = nc.dram_tensor("bo", [DIM], FP32, kind="ExternalInput")
    y = nc.dram_tensor("y", [NQ, DIM], FP32, kind="ExternalOutput")
    rdd = nc.dram_tensor("rdd", [4, 2, NQ], FP32)

    with tile.TileContext(nc) as tc, ExitStack() as ctx:
        _body(tc, ctx, c1s, c2b, wq, wk, wv, wo, bo, y, rdd)
    if os.environ.get("KERNEL_STRIP_WAITS", "1") == "1":
        _strip_pe_self_waits(nc)
        _strip_redundant_waits(nc)
    nc.finalize()
    return nc


def _body(tc, ctx, c1s, c2b, wq, wk, wv, wo, bo, y, rdd):
    nc = tc.nc
    Exp = mybir.ActivationFunctionType.Exp
    MULT = mybir.AluOpType.mult
    ADD = mybir.AluOpType.add

    persist = ctx.enter_context(tc.tile_pool(name="persist", bufs=1))
    stage = ctx.enter_context(tc.tile_pool(name="stage", bufs=1))

    # ---- constants / weights ----
    ident_gp = persist.tile([P, P], FP32, tag="ident_gp")
    make_identity(nc, ident_gp)
    ident = persist.tile([P, P], F32R, tag="ident")
    nc.vector.tensor_copy(out=ident, in_=ident_gp)

    # issue activation loads first -- transposes are the critical path
    c1nat = stage.tile([P, NQ // P, DIM], F32R, tag="cnat")
    c1r = c1s.rearrange("(n p) d -> p n d", p=P)
    for ch in range(2):
        nc.sync.dma_start(out=c1nat[:, 4 * ch:4 * ch + 4, :],
                          in_=c1r[:, 4 * ch:4 * ch + 4, :])
    c2nat = stage.tile([P, NK // P, DIM], F32R, tag="c2nat")
    c2r = c2b.rearrange("(n p) d -> p n d", p=P)
    for ch in range(4):
        nc.sync.dma_start(out=c2nat[:, 4 * ch:4 * ch + 4, :],
                          in_=c2r[:, 4 * ch:4 * ch + 4, :])

    wq_sb = persist.tile([P, 2, DIM], F32R, tag="wq")
    wk_sb = persist.tile([P, 2, DIM], F32R, tag="wk")
    wv_sb = persist.tile([P, 2, DIM], F32R, tag="wv")
    # Wo for K=128 head-stacked contraction: wo4[p, hg, f] = Wo[hg*128+p, f]
    wo4 = persist.tile([P, 2, DIM], BF16, tag="wo4")
    for wi, (w_dram, w_sb) in enumerate(((wq, wq_sb), (wk, wk_sb), (wv, wv_sb),
                                         (wo, wo4))):
        wst = stage.tile([P, 2, DIM], FP32, tag=f"wst{wi}", name=f"wst{wi}")
        nc.sync.dma_start(out=wst, in_=w_dram.rearrange("(c p) f -> p c f", p=P))
        nc.vector.tensor_copy(out=w_sb, in_=wst)
    # bias broadcast to all partitions (staged through DVE like the weights)
    bo_st = stage.tile([P, DIM], FP32, tag="bo_st")
    nc.gpsimd.dma_start(out=bo_st, in_=bo[:].partition_broadcast(P))
    bo_bc = persist.tile([P, DIM], FP32, tag="bo")
    nc.vector.tensor_copy(out=bo_bc, in_=bo_st)
    # warm the ACT exp table while the prologue runs
    exp_warm = persist.tile([1, 4], BF16, tag="exp_warm")
    nc.scalar.activation(out=exp_warm, in_=bo_bc[0:1, 0:4],
                         func=Exp, scale=float(SCALE))
    pt_pool = ctx.enter_context(tc.tile_pool(name="pt", bufs=6))
    small1 = ctx.enter_context(tc.tile_pool(name="small1", bufs=1))
    yout = ctx.enter_context(tc.tile_pool(name="yout", bufs=8))
    dn_pool = ctx.enter_context(tc.tile_pool(name="dn", bufs=2))

    # ---- persistent activations ----
    c1T = [persist.tile([P, NQ], F32R, tag=f"c1T{i}", name=f"c1T{i}") for i in range(2)]
    c2T = [persist.tile([P, NK], F32R, tag=f"c2T{i}", name=f"c2T{i}") for i in range(2)]
    qT = [persist.tile([P, NQ], F32R, tag=f"qT{i}", name=f"qT{i}") for i in range(2)]
    kT = [persist.tile([P, NK], F32R, tag=f"kT{i}", name=f"kT{i}") for i in range(2)]
    # v with fused ones column: [128, kb, h, 33]
    v4 = persist.tile([P, NK // P, H, D + 1], BF16, tag="v4")
    # normalized per-head outputs: 4 heads stacked per 128 partitions:
    # out_sb2[(h%4)*32 + d, h//4, q]
    out_sb2 = persist.tile([P, 2, NQ], BF16, tag="out_sb2")

    with tc.tile_pool(name="st_psum", bufs=2, space="PSUM") as st_psum, \
         tc.tile_pool(name="av_psum", bufs=2, space="PSUM") as av_psum:

        def c1tp(n, fh):
            tp = st_psum.tile([P, 1024], F32R, tag="st", name="tp")
            nc.tensor.transpose(tp[:, :P], c1nat[:, n, fh * P:(fh + 1) * P], ident)
            nc.vector.tensor_copy(out=c1T[fh][:, n * P:(n + 1) * P], in_=tp[:, :P])

        def c2tp(n, fh):
            tp = st_psum.tile([P, 1024], F32R, tag="st", name="tp")
            nc.tensor.transpose(tp[:, :P], c2nat[:, n, fh * P:(fh + 1) * P], ident)
            nc.vector.tensor_copy(out=c2T[fh][:, n * P:(n + 1) * P], in_=tp[:, :P])

        def qt_proj(fb, qb):
            pp = st_psum.tile([P, 1024], FP32, tag="st", name="pp")[:, :512]
            for c in range(2):
                nc.tensor.matmul(
                    pp, lhsT=wq_sb[:, c, fb * P:(fb + 1) * P],
                    rhs=c1T[c][:, qb * 512:(qb + 1) * 512],
                    start=(c == 0), stop=(c == 1),
                )
            nc.vector.tensor_copy(out=qT[fb][:, qb * 512:(qb + 1) * 512], in_=pp)

        def kt_proj(fb, nb):
            pp = st_psum.tile([P, 1024], FP32, tag="st", name="pp")[:, :512]
            for c in range(2):
                nc.tensor.matmul(
                    pp, lhsT=wk_sb[:, c, fb * P:(fb + 1) * P],
                    rhs=c2T[c][:, nb * 512:(nb + 1) * 512],
                    start=(c == 0), stop=(c == 1),
                )
            nc.vector.tensor_copy(out=kT[fb][:, nb * 512:(nb + 1) * 512], in_=pp)

        def v_proj(kb):
            pp = st_psum.tile([P, 1024], FP32, tag="st", name="pp")[:, :512]
            for c in range(2):
                nc.tensor.matmul(
                    pp[:, :DIM], lhsT=c2T[c][:, kb * P:(kb + 1) * P],
                    rhs=wv_sb[:, c, :], start=(c == 0), stop=(c == 1),
                )
            nc.vector.tensor_copy(
                out=v4[:, kb, :, 0:D],
                in_=pp[:, :DIM].rearrange("p (h d) -> p h d", d=D),
            )

        nc.gpsimd.memset(v4, 1.0)  # ones column; v copies overwrite cols 0..D-1

        # minimal pre-pair-0 prologue: exactly what pair 0's first units need
        for n in range(8):
            for fh in range(2):
                c1tp(n, fh)
        qt_proj(0, 0)
        qt_proj(0, 1)
        for n in range(4):
            for fh in range(2):
                c2tp(n, fh)
        kt_proj(0, 0)

        # deadline-scheduled leftover prologue work, injected into pair-0 units
        extras = {}

        def sched(u, fn, *a):
            extras.setdefault(u, []).append((fn, a))

        for kb in range(16):
            sched(2 * kb, v_proj, kb)             # needed by AV at unit 2*kb+2
        for b in range(4, 16):
            for fh in range(2):
                sched(b - 2, c2tp, b, fh)         # needed by kT0(b//4) & v(b)
        for nb in (1, 2, 3):
            sched(8 * nb - 2, kt_proj, 0, nb)     # needed by S^T kb=4nb (unit 8nb)
        sched(26, qt_proj, 1, 0)
        sched(27, qt_proj, 1, 1)
        for nb in range(4):
            sched(28 + nb, kt_proj, 1, nb)

        # ---- attention: head pairs; row-packed S^T, col-tiled AV ----
        for pr in range(4):
            h0 = 2 * pr
            ht = h0 // 4
            b0, b1 = (h0 % 4) * 32, (h0 % 4) * 32 + 32
            av = av_psum.tile([64 + D + 1, NQ], FP32, tag="av")
            pending = []

            def emit_av(ent):
                pt, kb, qh = ent
                for e in range(2):
                    nc.tensor.matmul(
                        av[64 * e:64 * e + D + 1, qh * 512:(qh + 1) * 512],
                        lhsT=v4[:, kb, h0 + e, :],
                        rhs=pt[:, e * 512:(e + 1) * 512],
                        start=(kb == 0), stop=(kb == NK // P - 1),
                        skip_group_check=True,
                    )

            units = [(kb, qh) for kb in range(NK // P) for qh in range(NQ // 512)]
            for u, (kb, qh) in enumerate(units):
                if pr == 0:
                    for fn, a in extras.get(u, []):
                        fn(*a)
                lhsT0 = kT[ht][b0:b0 + 32, kb * P:(kb + 1) * P]
                lhsT1 = kT[ht][b1:b1 + 32, kb * P:(kb + 1) * P]
                qs = slice(qh * 512, (qh + 1) * 512)
                st = st_psum.tile([P, 1024], FP32, tag="st")
                nc.tensor.matmul(
                    st[:, 0:512], lhsT=lhsT0, rhs=qT[ht][b0:b0 + 32, qs],
                    start=True, stop=True, tile_position=(b0, 0),
                )
                nc.tensor.matmul(
                    st[:, 512:1024], lhsT=lhsT1, rhs=qT[ht][b1:b1 + 32, qs],
                    start=True, stop=True, tile_position=(b1, 0),
                )
                pt = pt_pool.tile([P, 1024], BF16, tag="pt")
                nc.scalar.activation(out=pt, in_=st, func=Exp, scale=float(SCALE))
                pending.append((pt, kb, qh))
                if len(pending) > 2:
                    emit_av(pending.pop(0))
            for ent in pending:
                emit_av(ent)

            # ---- normalize: denominators -> SBUF (bit-trick ops cannot read
            # PE-accumulated PSUM); e=1 copy rides on ACT so the two copies
            # overlap; one reciprocal; DMA broadcast; 2 DVE multiplies ----
            dn_sb = dn_pool.tile([1, 2 * NQ], FP32, tag="dn", name="dn")
            nc.vector.tensor_copy(out=dn_sb[:, 0:NQ], in_=av[D:D + 1, :])
            nc.scalar.copy(out=dn_sb[:, NQ:2 * NQ],
                           in_=av[64 + D:64 + D + 1, :])
            nc.vector.reciprocal_approx_fast(out=dn_sb, in_=dn_sb)
            nc.gpsimd.dma_start(out=rdd[pr], in_=dn_sb)
            bc_sb = small1.tile([32, 2 * NQ], FP32, tag="bcs", name="bcs")
            nc.gpsimd.dma_start(
                out=bc_sb,
                in_=rdd[pr].rearrange("two q -> (two q)").partition_broadcast(32),
            )
            for e in range(2):
                h = h0 + e
                hb = (h % 4) * 32
                nc.vector.tensor_tensor(
                    out=out_sb2[hb:hb + 32, h // 4, :],
                    in0=av[64 * e:64 * e + D, :],
                    in1=bc_sb[:, e * NQ:(e + 1) * NQ],
                    op=MULT,
                )

    # ---- output projection + bias: K=128 contraction, 4 heads at once ----
    with tc.tile_pool(name="y_psum", bufs=2, space="PSUM") as y_psum:
        for qb in range(NQ // P):
            yp = y_psum.tile([P, 512], FP32, tag="y")
            for hg in range(2):
                nc.tensor.matmul(
                    yp[:, :DIM],
                    lhsT=out_sb2[:, hg, qb * P:(qb + 1) * P],
                    rhs=wo4[:, hg, :],
                    start=(hg == 0), stop=(hg == 1),
                )
            ys = yout.tile([P, DIM], FP32, tag="ys")
            nc.vector.tensor_tensor(out=ys, in0=yp[:, :DIM], in1=bo_bc, op=ADD)
            st_inst = nc.sync.dma_start(out=y[qb * P:(qb + 1) * P, :], in_=ys)
            nc._y_store_names = getattr(nc, "_y_store_names", []) + [st_inst.ins.name]


_NC_CACHE = None


def _get_nc():
    global _NC_CACHE
    if _NC_CACHE is None:
        _NC_CACHE = build_nc()
    return _NC_CACHE


def make_in_maps(c2, c1, Wq, Wk, Wv, Wo, bo):
    c1 = np.asarray(c1, np.float32)
    c2 = np.asarray(c2, np.float32)
    Wq, Wk, Wv, Wo, bo = (np.asarray(a, np.float32) for a in (Wq, Wk, Wv, Wo, bo))
    in_maps = []
    for core in range(N_CORES):
        b, qh = core // 2, core % 2
        in_maps.append({
            "c1s": np.ascontiguousarray(c1[b, qh * NQ:(qh + 1) * NQ, :]),
            "c2b": np.ascontiguousarray(c2[b]),
            "wq": Wq, "wk": Wk, "wv": Wv, "wo": Wo, "bo": bo,
        })
    return in_maps


def assemble(results):
    out = np.empty((4, 2 * NQ, DIM), np.float32)
    for core in range(N_CORES):
        b, qh = core // 2, core % 2
        out[b, qh * NQ:(qh + 1) * NQ, :] = results[core]["y"]
    return out


def run_spmd(inputs, trace=False, **kwargs):
    from concourse.bass_utils import run_bass_kernel_spmd

    nc = _get_nc()
    in_maps = make_in_maps(**inputs)
    res = run_bass_kernel_spmd(
        nc, in_maps, core_ids=list(range(N_CORES)), trace=trace, **kwargs
    )
    return assemble(res.results), res


def kernel(c2, c1, Wq, Wk, Wv, Wo, bo):
    out, _ = run_spmd(dict(c2=c2, c1=c1, Wq=Wq, Wk=Wk, Wv=Wv, Wo=Wo, bo=bo))
    return out


# revision 16
# speedup vs baseline: 1.0532x; 1.0019x over previous
"""Trainium2 Bass kernel for multi-head attention (b=4, n=2048, dim=256, H=8, D=32).

Sharding: 8 cores = 4 batches x 2 query-halves. Each core computes the full
attention for its 1024 query rows against all 2048 keys of its batch.
No collectives; host slices inputs and concatenates outputs.

Per-core dataflow (f32r storage, bf16 probabilities/values):
  c1s [1024,256], c2b [2048,256] --PE transpose--> c1T [256,1024], c2T [256,2048]
  qT = Wq^T c1^T  [256,1024]   (features on partitions; head h at 32h%128)
  kT = Wk^T c2^T  [256,2048]
  v  = c2 @ Wv    [2048, 8x(32+1)]  (keys on partitions; ones column per head)
  per unit (kb, qh): S^T = kT_h^T x qT_h  (f32r, PSUM-write-bound 1 col/cyc)
                     P^T = exp(0.125*S^T) -> bf16  (ACT)
  AV: two heads on disjoint PE column tiles (cols 0-63 / 64-127) so their
      rhs streams run concurrently on separate XBUSes (~1.4x aggregate)
  normalize: denominator rows -> SBUF (bit-trick DVE ops cannot read
      PE-accumulated PSUM), one reciprocal_approx_fast, DMA broadcast,
      2 DVE multiplies into out_sb2 (4 heads stacked per 128 partitions)
  y = out @ Wo: K=128 contraction (4 heads at once), 2 matmuls per q-block
"""

import os
import sys

for p in ("/opt/trn_rl_repo", "/opt/pypackages"):
    if p not in sys.path:
        sys.path.insert(0, p)

from contextlib import ExitStack

import numpy as np

import concourse.bass as bass
import concourse.bacc as bacc
import concourse.mybir as mybir
import concourse.tile as tile
from concourse.masks import make_identity

P = 128
NQ = 1024          # per-core query rows
NK = 2048          # keys
DIM = 256
H = 8
D = 32
SCALE = 64 ** -0.5  # 0.125, matches reference
FP32 = mybir.dt.float32
F32R = mybir.dt.float32r
BF16 = mybir.dt.bfloat16

N_CORES = 8


def _strip_pe_self_waits(nc):
    """Drop PE-sem waits from PE matmuls. The PE is strictly in-order with a
    single PSUM write port and never reads PSUM nor writes SBUF, so a PE
    instruction can never race another PE instruction; Tile still emits these
    same-engine waits, and matmul instructions only support one sync wait."""
    pe = mybir.EngineType.PE
    for f in nc.m.functions:
        for bb in f.blocks:
            for inst in bb.instructions:
                if type(inst).__name__ != "InstMatmult" or inst.engine != pe:
                    continue
                si = inst.sync_info
                if si is None:
                    continue
                ws = [w for w in si.on_wait if not str(w.ant_name).startswith("PE_")]
                if len(ws) != len(si.on_wait):
                    si.on_wait = ws
                    inst.sync_info = si


def _strip_redundant_waits(nc):
    """ACT is also strictly in-order: drop Activation-sem self-waits from
    ACTIVATE instructions (WAW on cycled SBUF output slots is FIFO-safe).
    Output stores: drop DMAHW lane-bookkeeping waits (they only order the
    store against an unrelated earlier input DMA that reused the same
    round-robin completion lane; the data dependency is the DVE wait)."""
    act = mybir.EngineType.Activation
    store_names = set(getattr(nc, "_y_store_names", ()))
    for f in nc.m.functions:
        for bb in f.blocks:
            for inst in bb.instructions:
                si = getattr(inst, "sync_info", None)
                if si is None or len(si.on_wait) <= 1:
                    continue
                tn = type(inst).__name__
                if tn == "InstActivation" and inst.engine == act:
                    ws = [w for w in si.on_wait
                          if not str(w.ant_name).startswith("Activation")]
                elif tn == "InstDMACopy" and inst.name in store_names:
                    ws = [w for w in si.on_wait
                          if not str(w.ant_name).startswith("DMAHW")]
                else:
                    continue
                if len(ws) != len(si.on_wait):
                    si.on_wait = ws
                    inst.sync_info = si


def build_nc():
    nc = bacc.Bacc()
    c1s = nc.dram_tensor("c1s", [NQ, DIM], F32R, kind="ExternalInput")
    c2b = nc.dram_tensor("c2b", [NK, DIM], F32R, kind="ExternalInput")
    wq = nc.dram_tensor("wq", [DIM, DIM], FP32, kind="ExternalInput")
    wk = nc.dram_tensor("wk", [DIM, DIM], FP32, kind="ExternalInput")
    wv = nc.dram_tensor("wv", [DIM, DIM], FP32, kind="ExternalInput")
    wo = nc.dram_tensor("wo", [DIM, DIM], FP32, kind="ExternalInput")
    bo = nc.dram_tensor("bo", [DIM], FP32, kind="ExternalInput")
    y = nc.dram_tensor("y", [NQ, DIM], FP32, kind="ExternalOutput")
    rdd = nc.dram_tensor("rdd", [4, 2, NQ], FP32)

    with tile.TileContext(nc) as tc, ExitStack() as ctx:
        _body(tc, ctx, c1s, c2b, wq, wk, wv, wo, bo, y, rdd)
    if os.environ.get("KERNEL_STRIP_WAITS", "1") == "1":
        _strip_pe_self_waits(nc)
        _strip_redundant_waits(nc)
    nc.finalize()
    return nc


def _body(tc, ctx, c1s, c2b, wq, wk, wv, wo, bo, y, rdd):
    nc = tc.nc
    Exp = mybir.ActivationFunctionType.Exp
    MULT = mybir.AluOpType.mult
    ADD = mybir.AluOpType.add

    persist = ctx.enter_context(tc.tile_pool(name="persist", bufs=1))
    stage = ctx.enter_context(tc.tile_pool(name="stage", bufs=1))

    # ---- constants / weights ----
    ident_gp = persist.tile([P, P], FP32, tag="ident_gp")
    make_identity(nc, ident_gp)
    ident = persist.tile([P, P], F32R, tag="ident")
    nc.vector.tensor_copy(out=ident, in_=ident_gp)

    # issue activation loads first -- transposes are the critical path
    c1nat = stage.tile([P, NQ // P, DIM], F32R, tag="cnat")
    c1r = c1s.rearrange("(n p) d -> p n d", p=P)
    for ch in range(2):
        nc.scalar.dma_start(out=c1nat[:, 4 * ch:4 * ch + 4, :],
                            in_=c1r[:, 4 * ch:4 * ch + 4, :])
    c2nat = stage.tile([P, NK // P, DIM], F32R, tag="c2nat")
    c2r = c2b.rearrange("(n p) d -> p n d", p=P)
    for ch in range(4):
        nc.sync.dma_start(out=c2nat[:, 4 * ch:4 * ch + 4, :],
                          in_=c2r[:, 4 * ch:4 * ch + 4, :])

    wq_sb = persist.tile([P, 2, DIM], F32R, tag="wq")
    wk_sb = persist.tile([P, 2, DIM], F32R, tag="wk")
    wv_sb = persist.tile([P, 2, DIM], F32R, tag="wv")
    # Wo for K=128 head-stacked contraction: wo4[p, hg, f] = Wo[hg*128+p, f]
    wo4 = persist.tile([P, 2, DIM], BF16, tag="wo4")
    for wi, (w_dram, w_sb) in enumerate(((wq, wq_sb), (wk, wk_sb), (wv, wv_sb),
                                         (wo, wo4))):
        wst = stage.tile([P, 2, DIM], FP32, tag=f"wst{wi}", name=f"wst{wi}")
        nc.scalar.dma_start(out=wst,
                            in_=w_dram.rearrange("(c p) f -> p c f", p=P))
        nc.vector.tensor_copy(out=w_sb, in_=wst)
    # bias broadcast to all partitions (staged through DVE like the weights)
    bo_st = stage.tile([P, DIM], FP32, tag="bo_st")
    nc.gpsimd.dma_start(out=bo_st, in_=bo[:].partition_broadcast(P))
    bo_bc = persist.tile([P, DIM], FP32, tag="bo")
    nc.vector.tensor_copy(out=bo_bc, in_=bo_st)
    # warm the ACT exp table while the prologue runs
    exp_warm = persist.tile([1, 4], BF16, tag="exp_warm")
    nc.scalar.activation(out=exp_warm, in_=bo_bc[0:1, 0:4],
                         func=Exp, scale=float(SCALE))
    pt_pool = ctx.enter_context(tc.tile_pool(name="pt", bufs=6))
    small1 = ctx.enter_context(tc.tile_pool(name="small1", bufs=1))
    yout = ctx.enter_context(tc.tile_pool(name="yout", bufs=8))
    dn_pool = ctx.enter_context(tc.tile_pool(name="dn", bufs=2))

    # ---- persistent activations ----
    c1T = [persist.tile([P, NQ], F32R, tag=f"c1T{i}", name=f"c1T{i}") for i in range(2)]
    c2T = [persist.tile([P, NK], F32R, tag=f"c2T{i}", name=f"c2T{i}") for i in range(2)]
    qT = [persist.tile([P, NQ], F32R, tag=f"qT{i}", name=f"qT{i}") for i in range(2)]
    kT = [persist.tile([P, NK], F32R, tag=f"kT{i}", name=f"kT{i}") for i in range(2)]
    # v with fused ones column: [128, kb, h, 33]
    v4 = persist.tile([P, NK // P, H, D + 1], BF16, tag="v4")
    # normalized per-head outputs: 4 heads stacked per 128 partitions:
    # out_sb2[(h%4)*32 + d, h//4, q]
    out_sb2 = persist.tile([P, 2, NQ], BF16, tag="out_sb2")

    with tc.tile_pool(name="st_psum", bufs=2, space="PSUM") as st_psum, \
         tc.tile_pool(name="av_psum", bufs=2, space="PSUM") as av_psum:

        def c1tp(n, fh):
            tp = st_psum.tile([P, 1024], F32R, tag="st", name="tp")
            nc.tensor.transpose(tp[:, :P], c1nat[:, n, fh * P:(fh + 1) * P], ident)
            nc.vector.tensor_copy(out=c1T[fh][:, n * P:(n + 1) * P], in_=tp[:, :P])

        def c2tp(n, fh):
            tp = st_psum.tile([P, 1024], F32R, tag="st", name="tp")
            nc.tensor.transpose(tp[:, :P], c2nat[:, n, fh * P:(fh + 1) * P], ident)
            nc.vector.tensor_copy(out=c2T[fh][:, n * P:(n + 1) * P], in_=tp[:, :P])

        def qt_proj(fb, qb):
            pp = st_psum.tile([P, 1024], FP32, tag="st", name="pp")[:, :512]
            for c in range(2):
                nc.tensor.matmul(
                    pp, lhsT=wq_sb[:, c, fb * P:(fb + 1) * P],
                    rhs=c1T[c][:, qb * 512:(qb + 1) * 512],
                    start=(c == 0), stop=(c == 1),
                )
            nc.vector.tensor_copy(out=qT[fb][:, qb * 512:(qb + 1) * 512], in_=pp)

        def kt_proj(fb, nb):
            pp = st_psum.tile([P, 1024], FP32, tag="st", name="pp")[:, :512]
            for c in range(2):
                nc.tensor.matmul(
                    pp, lhsT=wk_sb[:, c, fb * P:(fb + 1) * P],
                    rhs=c2T[c][:, nb * 512:(nb + 1) * 512],
                    start=(c == 0), stop=(c == 1),
                )
            nc.vector.tensor_copy(out=kT[fb][:, nb * 512:(nb + 1) * 512], in_=pp)

        def v_proj(kb):
            pp = st_psum.tile([P, 1024], FP32, tag="st", name="pp")[:, :512]
            for c in range(2):
                nc.tensor.matmul(
                    pp[:, :DIM], lhsT=c2T[c][:, kb * P:(kb + 1) * P],
                    rhs=wv_sb[:, c, :], start=(c == 0), stop=(c == 1),
                )
            nc.vector.tensor_copy(
                out=v4[:, kb, :, 0:D],
                in_=pp[:, :DIM].rearrange("p (h d) -> p h d", d=D),
            )

        nc.gpsimd.memset(v4, 1.0)  # ones column; v copies overwrite cols 0..D-1

        # minimal pre-pair-0 prologue: exactly what pair 0's first units need
        for n in range(8):
            for fh in range(2):
                c1tp(n, fh)
        qt_proj(0, 0)
        qt_proj(0, 1)
        for n in range(4):
            for fh in range(2):
                c2tp(n, fh)
        kt_proj(0, 0)

        # y partial for heads 0-3 (ready after pair 1) with fused bias
        yh0_sb = persist.tile([P, NQ // P, DIM], FP32, tag="yh0")

        def yh0(qb):
            pp = st_psum.tile([P, 1024], FP32, tag="st", name="pp")[:, :DIM]
            nc.tensor.matmul(
                pp, lhsT=out_sb2[:, 0, qb * P:(qb + 1) * P],
                rhs=wo4[:, 0, :], start=True, stop=True,
            )
            nc.vector.tensor_tensor(out=yh0_sb[:, qb, :], in0=pp, in1=bo_bc,
                                    op=ADD)

        # deadline-scheduled work injected into the unit stream
        extras = {}

        def sched(pu, fn, *a):
            extras.setdefault(pu, []).append((fn, a))

        for kb in range(16):
            sched((0, 2 * kb), v_proj, kb)        # needed by AV at unit 2*kb+2
        for b in range(4, 16):
            for fh in range(2):
                sched((0, b - 2), c2tp, b, fh)    # needed by kT0(b//4) & v(b)
        for nb in (1, 2, 3):
            sched((0, 8 * nb - 2), kt_proj, 0, nb)  # needed by S^T kb=4nb
        sched((0, 26), qt_proj, 1, 0)
        sched((0, 27), qt_proj, 1, 1)
        for nb in range(4):
            sched((0, 28 + nb), kt_proj, 1, nb)
        for qb in range(NQ // P):
            sched((2, 12 + 2 * qb), yh0, qb)      # heads 0-3 done after pair 1

        # ---- attention: head pairs; row-packed S^T, col-tiled AV ----
        for pr in range(4):
            h0 = 2 * pr
            ht = h0 // 4
            b0, b1 = (h0 % 4) * 32, (h0 % 4) * 32 + 32
            av = av_psum.tile([64 + D + 1, NQ], FP32, tag="av")
            pending = []

            def emit_av(ent):
                pt, kb, qh = ent
                for e in range(2):
                    nc.tensor.matmul(
                        av[64 * e:64 * e + D + 1, qh * 512:(qh + 1) * 512],
                        lhsT=v4[:, kb, h0 + e, :],
                        rhs=pt[:, e * 512:(e + 1) * 512],
                        start=(kb == 0), stop=(kb == NK // P - 1),
                        skip_group_check=True,
                    )

            units = [(kb, qh) for kb in range(NK // P) for qh in range(NQ // 512)]
            for u, (kb, qh) in enumerate(units):
                for fn, a in extras.get((pr, u), []):
                    fn(*a)
                lhsT0 = kT[ht][b0:b0 + 32, kb * P:(kb + 1) * P]
                lhsT1 = kT[ht][b1:b1 + 32, kb * P:(kb + 1) * P]
                qs = slice(qh * 512, (qh + 1) * 512)
                st = st_psum.tile([P, 1024], FP32, tag="st")
                nc.tensor.matmul(
                    st[:, 0:512], lhsT=lhsT0, rhs=qT[ht][b0:b0 + 32, qs],
                    start=True, stop=True, tile_position=(b0, 0),
                )
                nc.tensor.matmul(
                    st[:, 512:1024], lhsT=lhsT1, rhs=qT[ht][b1:b1 + 32, qs],
                    start=True, stop=True, tile_position=(b1, 0),
                )
                pt = pt_pool.tile([P, 1024], BF16, tag="pt")
                nc.scalar.activation(out=pt, in_=st, func=Exp, scale=float(SCALE))
                pending.append((pt, kb, qh))
                if len(pending) > 2:
                    emit_av(pending.pop(0))
            for ent in pending:
                emit_av(ent)

            # ---- normalize: denominators -> SBUF (bit-trick ops cannot read
            # PE-accumulated PSUM); e=1 copy rides on ACT so the two copies
            # overlap; one reciprocal; DMA broadcast; 2 DVE multiplies ----
            dn_sb = dn_pool.tile([1, 2 * NQ], FP32, tag="dn", name="dn")
            nc.vector.tensor_copy(out=dn_sb[:, 0:NQ], in_=av[D:D + 1, :])
            nc.scalar.copy(out=dn_sb[:, NQ:2 * NQ],
                           in_=av[64 + D:64 + D + 1, :])
            nc.vector.reciprocal_approx_fast(out=dn_sb, in_=dn_sb)
            bc_sb = small1.tile([32, 2 * NQ], FP32, tag="bcs", name="bcs")
            nc.gpsimd.partition_broadcast(bc_sb, dn_sb)
            for e in range(2):
                h = h0 + e
                hb = (h % 4) * 32
                nc.vector.tensor_tensor(
                    out=out_sb2[hb:hb + 32, h // 4, :],
                    in0=av[64 * e:64 * e + D, :],
                    in1=bc_sb[:, e * NQ:(e + 1) * NQ],
                    op=MULT,
                )

    # ---- output projection tail: heads 4-7 matmul + stored hg0 partial ----
    with tc.tile_pool(name="y_psum", bufs=2, space="PSUM") as y_psum:
        for qb in range(NQ // P):
            yp = y_psum.tile([P, 512], FP32, tag="y")
            nc.tensor.matmul(
                yp[:, :DIM],
                lhsT=out_sb2[:, 1, qb * P:(qb + 1) * P],
                rhs=wo4[:, 1, :], start=True, stop=True,
            )
            ys = yout.tile([P, DIM], FP32, tag="ys")
            nc.vector.tensor_tensor(out=ys, in0=yp[:, :DIM],
                                    in1=yh0_sb[:, qb, :], op=ADD)
            st_inst = nc.sync.dma_start(out=y[qb * P:(qb + 1) * P, :], in_=ys)
            nc._y_store_names = getattr(nc, "_y_store_names", []) + [st_inst.ins.name]


_NC_CACHE = None


def _get_nc():
    global _NC_CACHE
    if _NC_CACHE is None:
        _NC_CACHE = build_nc()
    return _NC_CACHE


def make_in_maps(c2, c1, Wq, Wk, Wv, Wo, bo):
    c1 = np.asarray(c1, np.float32)
    c2 = np.asarray(c2, np.float32)
    Wq, Wk, Wv, Wo, bo = (np.asarray(a, np.float32) for a in (Wq, Wk, Wv, Wo, bo))
    in_maps = []
    for core in range(N_CORES):
        b, qh = core // 2, core % 2
        in_maps.append({
            "c1s": np.ascontiguousarray(c1[b, qh * NQ:(qh + 1) * NQ, :]),
            "c2b": np.ascontiguousarray(c2[b]),
            "wq": Wq, "wk": Wk, "wv": Wv, "wo": Wo, "bo": bo,
        })
    return in_maps


def assemble(results):
    out = np.empty((4, 2 * NQ, DIM), np.float32)
    for core in range(N_CORES):
        b, qh = core // 2, core % 2
        out[b, qh * NQ:(qh + 1) * NQ, :] = results[core]["y"]
    return out


def run_spmd(inputs, trace=False, **kwargs):
    from concourse.bass_utils import run_bass_kernel_spmd

    nc = _get_nc()
    in_maps = make_in_maps(**inputs)
    res = run_bass_kernel_spmd(
        nc, in_maps, core_ids=list(range(N_CORES)), trace=trace, **kwargs
    )
    return assemble(res.results), res


def kernel(c2, c1, Wq, Wk, Wv, Wo, bo):
    out, _ = run_spmd(dict(c2=c2, c1=c1, Wq=Wq, Wk=Wk, Wv=Wv, Wo=Wo, bo=bo))
    return out


# revision 18
# speedup vs baseline: 1.0569x; 1.0035x over previous
"""Trainium2 Bass kernel for multi-head attention (b=4, n=2048, dim=256, H=8, D=32).

Sharding: 8 cores = 4 batches x 2 query-halves. Each core computes the full
attention for its 1024 query rows against all 2048 keys of its batch.
No collectives; host slices inputs and concatenates outputs.

Per-core dataflow (f32r storage, bf16 probabilities/values):
  c1s [1024,256], c2b [2048,256] --PE transpose--> c1T [256,1024], c2T [256,2048]
  qT = Wq^T c1^T  [256,1024]   (features on partitions; head h at 32h%128)
  kT = Wk^T c2^T  [256,2048]
  v  = c2 @ Wv    [2048, 8x(32+1)]  (keys on partitions; ones column per head)
  per unit (kb, qh): S^T = kT_h^T x qT_h  (f32r, PSUM-write-bound 1 col/cyc)
                     P^T = exp(0.125*S^T) -> bf16  (ACT)
  AV: two heads on disjoint PE column tiles (cols 0-63 / 64-127) so their
      rhs streams run concurrently on separate XBUSes (~1.4x aggregate)
  normalize: denominator rows -> SBUF (bit-trick DVE ops cannot read
      PE-accumulated PSUM), one reciprocal_approx_fast, DMA broadcast,
      2 DVE multiplies into out_sb2 (4 heads stacked per 128 partitions)
  y = out @ Wo: K=128 contraction (4 heads at once), 2 matmuls per q-block
"""

import os
import sys

for p in ("/opt/trn_rl_repo", "/opt/pypackages"):
    if p not in sys.path:
        sys.path.insert(0, p)

from contextlib import ExitStack

import numpy as np

import concourse.bass as bass
import concourse.bacc as bacc
import concourse.mybir as mybir
import concourse.tile as tile
from concourse.masks import make_identity

P = 128
NQ = 1024          # per-core query rows
NK = 2048          # keys
DIM = 256
H = 8
D = 32
SCALE = 64 ** -0.5  # 0.125, matches reference
FP32 = mybir.dt.float32
F32R = mybir.dt.float32r
BF16 = mybir.dt.bfloat16

N_CORES = 8


def _strip_pe_self_waits(nc):
    """Drop PE-sem waits from PE matmuls. The PE is strictly in-order with a
    single PSUM write port and never reads PSUM nor writes SBUF, so a PE
    instruction can never race another PE instruction; Tile still emits these
    same-engine waits, and matmul instructions only support one sync wait."""
    pe = mybir.EngineType.PE
    for f in nc.m.functions:
        for bb in f.blocks:
            for inst in bb.instructions:
                if type(inst).__name__ != "InstMatmult" or inst.engine != pe:
                    continue
                si = inst.sync_info
                if si is None:
                    continue
                ws = [w for w in si.on_wait if not str(w.ant_name).startswith("PE_")]
                if len(ws) != len(si.on_wait):
                    si.on_wait = ws
                    inst.sync_info = si


def _strip_redundant_waits(nc):
    """ACT is also strictly in-order: drop Activation-sem self-waits from
    ACTIVATE instructions (WAW on cycled SBUF output slots is FIFO-safe).
    Output stores: drop DMAHW lane-bookkeeping waits (they only order the
    store against an unrelated earlier input DMA that reused the same
    round-robin completion lane; the data dependency is the DVE wait)."""
    act = mybir.EngineType.Activation
    store_names = set(getattr(nc, "_y_store_names", ()))
    for f in nc.m.functions:
        for bb in f.blocks:
            for inst in bb.instructions:
                si = getattr(inst, "sync_info", None)
                if si is None or len(si.on_wait) <= 1:
                    continue
                tn = type(inst).__name__
                if tn == "InstActivation" and inst.engine == act:
                    ws = [w for w in si.on_wait
                          if not str(w.ant_name).startswith("Activation")]
                elif tn == "InstDMACopy" and inst.name in store_names:
                    ws = [w for w in si.on_wait
                          if not str(w.ant_name).startswith("DMAHW")]
                else:
                    continue
                if len(ws) != len(si.on_wait):
                    si.on_wait = ws
                    inst.sync_info = si


def build_nc():
    nc = bacc.Bacc()
    c1s = nc.dram_tensor("c1s", [NQ, DIM], F32R, kind="ExternalInput")
    c2b = nc.dram_tensor("c2b", [NK, DIM], F32R, kind="ExternalInput")
    wq = nc.dram_tensor("wq", [DIM, DIM], FP32, kind="ExternalInput")
    wk = nc.dram_tensor("wk", [DIM, DIM], FP32, kind="ExternalInput")
    wv = nc.dram_tensor("wv", [DIM, DIM], FP32, kind="ExternalInput")
    wo = nc.dram_tensor("wo", [DIM, DIM], FP32, kind="ExternalInput")
    bo = nc.dram_tensor("bo", [DIM], FP32, kind="ExternalInput")
    y = nc.dram_tensor("y", [NQ, DIM], FP32, kind="ExternalOutput")
    rdd = nc.dram_tensor("rdd", [4, 2, NQ], FP32)

    with tile.TileContext(nc) as tc, ExitStack() as ctx:
        _body(tc, ctx, c1s, c2b, wq, wk, wv, wo, bo, y, rdd)
    if os.environ.get("KERNEL_STRIP_WAITS", "1") == "1":
        _strip_pe_self_waits(nc)
        _strip_redundant_waits(nc)
    nc.finalize()
    return nc


def _body(tc, ctx, c1s, c2b, wq, wk, wv, wo, bo, y, rdd):
    nc = tc.nc
    Exp = mybir.ActivationFunctionType.Exp
    MULT = mybir.AluOpType.mult
    ADD = mybir.AluOpType.add

    persist = ctx.enter_context(tc.tile_pool(name="persist", bufs=1))
    stage = ctx.enter_context(tc.tile_pool(name="stage", bufs=1))

    # ---- constants / weights ----
    ident_gp = persist.tile([P, P], FP32, tag="ident_gp")
    make_identity(nc, ident_gp)
    ident = persist.tile([P, P], F32R, tag="ident")
    nc.vector.tensor_copy(out=ident, in_=ident_gp)

    # issue activation loads first -- transposes are the critical path
    c1nat = stage.tile([P, NQ // P, DIM], F32R, tag="cnat")
    c1r = c1s.rearrange("(n p) d -> p n d", p=P)
    for ch in range(4):
        nc.scalar.dma_start(out=c1nat[:, 2 * ch:2 * ch + 2, :],
                            in_=c1r[:, 2 * ch:2 * ch + 2, :])
    c2nat = stage.tile([P, NK // P, DIM], F32R, tag="c2nat")
    c2r = c2b.rearrange("(n p) d -> p n d", p=P)
    for ch in range(8):
        nc.sync.dma_start(out=c2nat[:, 2 * ch:2 * ch + 2, :],
                          in_=c2r[:, 2 * ch:2 * ch + 2, :])

    wq_sb = persist.tile([P, 2, DIM], F32R, tag="wq")
    wk_sb = persist.tile([P, 2, DIM], F32R, tag="wk")
    wv_sb = persist.tile([P, 2, DIM], F32R, tag="wv")
    # Wo for K=128 head-stacked contraction: wo4[p, hg, f] = Wo[hg*128+p, f]
    wo4 = persist.tile([P, 2, DIM], BF16, tag="wo4")
    for wi, (w_dram, w_sb) in enumerate(((wq, wq_sb), (wk, wk_sb), (wv, wv_sb),
                                         (wo, wo4))):
        wst = stage.tile([P, 2, DIM], FP32, tag=f"wst{wi}", name=f"wst{wi}")
        nc.scalar.dma_start(out=wst,
                            in_=w_dram.rearrange("(c p) f -> p c f", p=P))
        nc.vector.tensor_copy(out=w_sb, in_=wst)
    # bias broadcast to all partitions (staged through DVE like the weights)
    bo_st = stage.tile([P, DIM], FP32, tag="bo_st")
    nc.gpsimd.dma_start(out=bo_st, in_=bo[:].partition_broadcast(P))
    bo_bc = persist.tile([P, DIM], FP32, tag="bo")
    nc.vector.tensor_copy(out=bo_bc, in_=bo_st)
    # warm the ACT exp table while the prologue runs
    exp_warm = persist.tile([1, 4], BF16, tag="exp_warm")
    nc.scalar.activation(out=exp_warm, in_=bo_bc[0:1, 0:4],
                         func=Exp, scale=float(SCALE))
    pt_pool = ctx.enter_context(tc.tile_pool(name="pt", bufs=8))
    small1 = ctx.enter_context(tc.tile_pool(name="small1", bufs=1))
    yout = ctx.enter_context(tc.tile_pool(name="yout", bufs=8))
    dn_pool = ctx.enter_context(tc.tile_pool(name="dn", bufs=2))

    # ---- persistent activations ----
    c1T = [persist.tile([P, NQ], F32R, tag=f"c1T{i}", name=f"c1T{i}") for i in range(2)]
    c2T = [persist.tile([P, NK], F32R, tag=f"c2T{i}", name=f"c2T{i}") for i in range(2)]
    qT = [persist.tile([P, NQ], F32R, tag=f"qT{i}", name=f"qT{i}") for i in range(2)]
    kT = [persist.tile([P, NK], F32R, tag=f"kT{i}", name=f"kT{i}") for i in range(2)]
    # v with fused ones column: [128, kb, h, 33]
    v4 = persist.tile([P, NK // P, H, D + 1], BF16, tag="v4")
    # normalized per-head outputs: 4 heads stacked per 128 partitions:
    # out_sb2[(h%4)*32 + d, h//4, q]
    out_sb2 = persist.tile([P, 2, NQ], BF16, tag="out_sb2")

    with tc.tile_pool(name="st_psum", bufs=2, space="PSUM") as st_psum, \
         tc.tile_pool(name="av_psum", bufs=2, space="PSUM") as av_psum:

        def c1tp(n, fh):
            tp = st_psum.tile([P, 1024], F32R, tag="st", name="tp")
            nc.tensor.transpose(tp[:, :P], c1nat[:, n, fh * P:(fh + 1) * P], ident)
            nc.vector.tensor_copy(out=c1T[fh][:, n * P:(n + 1) * P], in_=tp[:, :P])

        def c2tp(n, fh):
            tp = st_psum.tile([P, 1024], F32R, tag="st", name="tp")
            nc.tensor.transpose(tp[:, :P], c2nat[:, n, fh * P:(fh + 1) * P], ident)
            nc.vector.tensor_copy(out=c2T[fh][:, n * P:(n + 1) * P], in_=tp[:, :P])

        def qt_proj(fb, qb):
            pp = st_psum.tile([P, 1024], FP32, tag="st", name="pp")[:, :512]
            for c in range(2):
                nc.tensor.matmul(
                    pp, lhsT=wq_sb[:, c, fb * P:(fb + 1) * P],
                    rhs=c1T[c][:, qb * 512:(qb + 1) * 512],
                    start=(c == 0), stop=(c == 1),
                )
            nc.vector.tensor_copy(out=qT[fb][:, qb * 512:(qb + 1) * 512], in_=pp)

        def kt_proj(fb, nb):
            pp = st_psum.tile([P, 1024], FP32, tag="st", name="pp")[:, :512]
            for c in range(2):
                nc.tensor.matmul(
                    pp, lhsT=wk_sb[:, c, fb * P:(fb + 1) * P],
                    rhs=c2T[c][:, nb * 512:(nb + 1) * 512],
                    start=(c == 0), stop=(c == 1),
                )
            nc.vector.tensor_copy(out=kT[fb][:, nb * 512:(nb + 1) * 512], in_=pp)

        def v_proj(kb):
            pp = st_psum.tile([P, 1024], FP32, tag="st", name="pp")[:, :512]
            for c in range(2):
                nc.tensor.matmul(
                    pp[:, :DIM], lhsT=c2T[c][:, kb * P:(kb + 1) * P],
                    rhs=wv_sb[:, c, :], start=(c == 0), stop=(c == 1),
                )
            nc.vector.tensor_copy(
                out=v4[:, kb, :, 0:D],
                in_=pp[:, :DIM].rearrange("p (h d) -> p h d", d=D),
            )

        nc.gpsimd.memset(v4, 1.0)  # ones column; v copies overwrite cols 0..D-1

        # minimal pre-pair-0 prologue: exactly what pair 0's first units need
        for n in range(8):
            for fh in range(2):
                c1tp(n, fh)
        qt_proj(0, 0)
        qt_proj(0, 1)
        for n in range(4):
            for fh in range(2):
                c2tp(n, fh)
        kt_proj(0, 0)

        # y partial for heads 0-3 (ready after pair 1) with fused bias
        yh0_sb = persist.tile([P, NQ // P, DIM], FP32, tag="yh0")

        def yh0(qb):
            pp = st_psum.tile([P, 1024], FP32, tag="st", name="pp")[:, :DIM]
            nc.tensor.matmul(
                pp, lhsT=out_sb2[:, 0, qb * P:(qb + 1) * P],
                rhs=wo4[:, 0, :], start=True, stop=True,
            )
            nc.vector.tensor_tensor(out=yh0_sb[:, qb, :], in0=pp, in1=bo_bc,
                                    op=ADD)

        # deadline-scheduled work injected into the unit stream
        extras = {}

        def sched(pu, fn, *a):
            extras.setdefault(pu, []).append((fn, a))

        for kb in range(16):
            sched((0, 2 * kb), v_proj, kb)        # needed by AV at unit 2*kb+2
        for b in range(4, 16):
            for fh in range(2):
                sched((0, b - 2), c2tp, b, fh)    # needed by kT0(b//4) & v(b)
        for nb in (1, 2, 3):
            sched((0, 8 * nb - 2), kt_proj, 0, nb)  # needed by S^T kb=4nb
        sched((0, 26), qt_proj, 1, 0)
        sched((0, 27), qt_proj, 1, 1)
        for nb in range(4):
            sched((0, 28 + nb), kt_proj, 1, nb)
        for qb in range(NQ // P):
            sched((2, 12 + 2 * qb), yh0, qb)      # heads 0-3 done after pair 1

        # ---- attention: head pairs; row-packed S^T, col-tiled AV ----
        for pr in range(4):
            h0 = 2 * pr
            ht = h0 // 4
            b0, b1 = (h0 % 4) * 32, (h0 % 4) * 32 + 32
            av = av_psum.tile([64 + D + 1, NQ], FP32, tag="av")
            pending = []

            def emit_av(ent):
                pt, kb, qh = ent
                for e in range(2):
                    nc.tensor.matmul(
                        av[64 * e:64 * e + D + 1, qh * 512:(qh + 1) * 512],
                        lhsT=v4[:, kb, h0 + e, :],
                        rhs=pt[:, e * 512:(e + 1) * 512],
                        start=(kb == 0), stop=(kb == NK // P - 1),
                        skip_group_check=True,
                    )

            units = [(kb, qh) for kb in range(NK // P) for qh in range(NQ // 512)]
            for u, (kb, qh) in enumerate(units):
                for fn, a in extras.get((pr, u), []):
                    fn(*a)
                lhsT0 = kT[ht][b0:b0 + 32, kb * P:(kb + 1) * P]
                lhsT1 = kT[ht][b1:b1 + 32, kb * P:(kb + 1) * P]
                qs = slice(qh * 512, (qh + 1) * 512)
                st = st_psum.tile([P, 1024], FP32, tag="st")
                nc.tensor.matmul(
                    st[:, 0:512], lhsT=lhsT0, rhs=qT[ht][b0:b0 + 32, qs],
                    start=True, stop=True, tile_position=(b0, 0),
                )
                nc.tensor.matmul(
                    st[:, 512:1024], lhsT=lhsT1, rhs=qT[ht][b1:b1 + 32, qs],
                    start=True, stop=True, tile_position=(b1, 0),
                )
                pt = pt_pool.tile([P, 1024], BF16, tag="pt")
                nc.scalar.activation(out=pt, in_=st, func=Exp, scale=float(SCALE))
                pending.append((pt, kb, qh))
                if len(pending) > 2:
                    emit_av(pending.pop(0))
            for ent in pending:
                emit_av(ent)

            # ---- normalize: denominators -> SBUF (bit-trick ops cannot read
            # PE-accumulated PSUM); e=1 copy rides on ACT so the two copies
            # overlap; one reciprocal; DMA broadcast; 2 DVE multiplies ----
            dn_sb = dn_pool.tile([1, 2 * NQ], FP32, tag="dn", name="dn")
            nc.vector.tensor_copy(out=dn_sb[:, 0:NQ], in_=av[D:D + 1, :])
            nc.scalar.copy(out=dn_sb[:, NQ:2 * NQ],
                           in_=av[64 + D:64 + D + 1, :])
            nc.vector.reciprocal_approx_fast(out=dn_sb, in_=dn_sb)
            bc_sb = small1.tile([32, 2 * NQ], FP32, tag="bcs", name="bcs")
            nc.gpsimd.partition_broadcast(bc_sb, dn_sb)
            for e in range(2):
                h = h0 + e
                hb = (h % 4) * 32
                nc.vector.tensor_tensor(
                    out=out_sb2[hb:hb + 32, h // 4, :],
                    in0=av[64 * e:64 * e + D, :],
                    in1=bc_sb[:, e * NQ:(e + 1) * NQ],
                    op=MULT,
                )

    # ---- output projection tail: heads 4-7 matmul + stored hg0 partial ----
    with tc.tile_pool(name="y_psum", bufs=2, space="PSUM") as y_psum:
        for qb in range(NQ // P):
            yp = y_psum.tile([P, 512], FP32, tag="y")
            nc.tensor.matmul(
                yp[:, :DIM],
                lhsT=out_sb2[:, 1, qb * P:(qb + 1) * P],
                rhs=wo4[:, 1, :], start=True, stop=True,
            )
            ys = yout.tile([P, DIM], FP32, tag="ys")
            nc.vector.tensor_tensor(out=ys, in0=yp[:, :DIM],
                                    in1=yh0_sb[:, qb, :], op=ADD)
            st_inst = nc.sync.dma_start(out=y[qb * P:(qb + 1) * P, :], in_=ys)
            nc._y_store_names = getattr(nc, "_y_store_names", []) + [st_inst.ins.name]


_NC_CACHE = None


def _get_nc():
    global _NC_CACHE
    if _NC_CACHE is None:
        _NC_CACHE = build_nc()
    return _NC_CACHE


def make_in_maps(c2, c1, Wq, Wk, Wv, Wo, bo):
    c1 = np.asarray(c1, np.float32)
    c2 = np.asarray(c2, np.float32)
    Wq, Wk, Wv, Wo, bo = (np.asarray(a, np.float32) for a in (Wq, Wk, Wv, Wo, bo))
    in_maps = []
    for core in range(N_CORES):
        b, qh = core // 2, core % 2
        in_maps.append({
            "c1s": np.ascontiguousarray(c1[b, qh * NQ:(qh + 1) * NQ, :]),
            "c2b": np.ascontiguousarray(c2[b]),
            "wq": Wq, "wk": Wk, "wv": Wv, "wo": Wo, "bo": bo,
        })
    return in_maps


def assemble(results):
    out = np.empty((4, 2 * NQ, DIM), np.float32)
    for core in range(N_CORES):
        b, qh = core // 2, core % 2
        out[b, qh * NQ:(qh + 1) * NQ, :] = results[core]["y"]
    return out


def run_spmd(inputs, trace=False, **kwargs):
    from concourse.bass_utils import run_bass_kernel_spmd

    nc = _get_nc()
    in_maps = make_in_maps(**inputs)
    res = run_bass_kernel_spmd(
        nc, in_maps, core_ids=list(range(N_CORES)), trace=trace, **kwargs
    )
    return assemble(res.results), res


def kernel(c2, c1, Wq, Wk, Wv, Wo, bo):
    out, _ = run_spmd(dict(c2=c2, c1=c1, Wq=Wq, Wk=Wk, Wv=Wv, Wo=Wo, bo=bo))
    return out
